# revision 61
# baseline (speedup 1.0000x reference)
"""Multi-head causal attention (B=2, S=2048, H=16, DH=64, D=1024) on 8 TRN2 cores.

Sharding: Megatron tensor-parallel over heads - core c owns heads {2c, 2c+1}:
  * column-slices of Wq/Wk/Wv (128 cols each) + bias slices,
  * row-slice of Wo (128 rows),
  * full hidden_states (pre-transposed on host to [D, B*S]).
Each core computes a partial output (its 2 heads through Wo rows); host sums
the 8 f16 partials (row-parallel unshard) and adds bo.

Device dataflow per core, interleaved so PE (matmul), ACT (exp) and DVE
(evacuations) overlap across phases; 512-token groups alternate batches so
attention blocks unlock progressively:
  per group g (b, j):
  A) QKV: qT/kT/vT [128, 512] = W_slice.T @ hiddenT chunk (contraction over D
     in 8 chunks of 128, f32 PSUM accumulate; bias added on evacuation).
     vT columns transposed into V_aug [tok128, chunk, head, 128] via DMA-xbar
     transpose; V_aug cols 64:128 are pre-set to ones so AV matmuls emit the
     softmax denominator replicated on PSUM partitions 64:127.
  B) attention blocks (b, 2j) and (b, 2j+1), both heads, causal:
     scoresT[kv,q] = kT_slice.T @ qT_slice (K=64, N=256, the two heads run
     concurrently on PE row-groups 0-63/64-127), exp via ACT straight off
     f16 PSUM, diagonal masked by triangular multiplies on GPSIMD, then
     ct[128, q] += V_aug.T @ expT (rows 0:64 ctx, 64:128 denominator).
     Normalize: DVE reciprocal of rows 64:128, multiply into ctxT.
  C) output projection for the finished 512 tokens: f16 PSUM matmul, DVE
     copy to SBUF, DMA to DRAM in f16.
"""
import os
import sys

sys.path.insert(0, "/opt/trn_rl_repo")

from contextlib import ExitStack

import numpy as np

import concourse.bass as bass
import concourse.mybir as mybir
import concourse.tile as tile
from concourse import bacc
from concourse.bass_utils import run_bass_kernel_spmd

F32 = mybir.dt.float32
F16 = mybir.dt.float16
MM_DT = F16
MM_NP = np.float16

B, S, H, DH = 2, 2048, 16, 64
D = H * DH            # 1024
T = B * S             # 4096 tokens
NCORES = 8
HPC = H // NCORES     # 2 heads per core
KC = D // 128         # 8 contraction chunks
NG = T // 512         # 8 token groups
GPB = NG // B         # 4 groups per batch
NKV = T // 128        # 32 kv chunks of 128 tokens
EXPFN = mybir.ActivationFunctionType.Exp

# DMA-xbar transpose serializes the whole HWDGE ring under Tile's
# hang-workaround (measured ~1.2us each + completion waits) - PE default.
USE_DMAT = os.environ.get("KERNEL_DMAT") == "1"
SP_DT = F32  # matmul outputs must be fp32 in PSUM


def _body(nc, tc, ctx, t_in, t_out, t_out_dbg=None):
    xt, wq, wk, wv, wo, bqkv, trid = t_in
    po = t_out

    const = ctx.enter_context(tc.tile_pool(name="const", bufs=1))
    big = ctx.enter_context(tc.tile_pool(name="big", bufs=1))
    xtp = ctx.enter_context(tc.tile_pool(name="xtp", bufs=4))
    ep = ctx.enter_context(tc.tile_pool(name="ep", bufs=8))
    rp = ctx.enter_context(tc.tile_pool(name="rp", bufs=4))
    osp = ctx.enter_context(tc.tile_pool(name="osp", bufs=4))
    vsp = ctx.enter_context(tc.tile_pool(name="vsp", bufs=3))

    # PSUM budget (8 banks):
    #   sp f32 2 banks x2 + ct0/ct1 1 bank x1 each + accop 1 bank x2 = 8
    ct_bufs = 1
    psS = ctx.enter_context(tc.tile_pool(name="psS", bufs=2, space="PSUM"))
    psC = ctx.enter_context(tc.tile_pool(name="psC", bufs=ct_bufs, space="PSUM"))
    psA = ctx.enter_context(tc.tile_pool(name="psA", bufs=2, space="PSUM"))

    # ---- constants / weights in SBUF
    wq_s = const.tile([128, KC, 128], MM_DT, tag="wq")
    wk_s = const.tile([128, KC, 128], MM_DT, tag="wk")
    wv_s = const.tile([128, KC, 128], MM_DT, tag="wv")
    wo_s = const.tile([128, D], MM_DT, tag="wo")
    bqkv_s = const.tile([128, 3], F32, tag="bqkv")
    trid_s = const.tile([128, 2, 128], MM_DT, tag="trid")
    bq_s, bk_s, bv_s = (bqkv_s[:, 0:1], bqkv_s[:, 1:2], bqkv_s[:, 2:3])
    tri_s, id_s = trid_s[:, 0, :], trid_s[:, 1, :]
    qT = big.tile([128, T], MM_DT, tag="qT")
    kT = big.tile([128, T], MM_DT, tag="kT")
    vT = big.tile([128, T], MM_DT, tag="vT")
    ctxT = big.tile([128, T], MM_DT, tag="ctxT")
    vaug = big.tile([128, NKV, HPC, 128], MM_DT, tag="vaug")

    woc = wo_s[:].rearrange("p (c n) -> p c n", c=KC)
    # partition-major view of po for single-DMA stores: [128, KC, T]
    po_pview = po.rearrange("c p t -> p c t")

    if t_out_dbg is not None:
        dbgp = ctx.enter_context(tc.tile_pool(name="dbgp", bufs=1))
        dbg_craw_s = dbgp.tile([128, T], F32, tag="craw")
        dbg_den_s = dbgp.tile([128, T], F32, tag="den")

    # groups alternate batches so attention work unlocks early and evenly
    steps = [(b, j) for j in range(GPB) for b in range(B)]

    xgs = {}

    def emit_xg_load(idx, engine=None, split=False):
        # prefetch the hidden-state slice for step idx (gpsimd/SWDGE ring so
        # it is not queued behind the po stores on the sync ring); split=True
        # halves the transfer so the first chain can start on the first half
        gg_l = steps[idx][0] * GPB + steps[idx][1]
        xg = xtp.tile([128, KC, 512], MM_DT, tag="xt")
        eng = engine or nc.gpsimd
        if split:
            eng.dma_start(xg[:, 0:KC // 2, :], xt[gg_l][:, 0:KC // 2, :])
            eng.dma_start(xg[:, KC // 2:, :], xt[gg_l][:, KC // 2:, :])
        else:
            eng.dma_start(xg[:], xt[gg_l])
        xgs[idx] = xg

    def qkv_quanta(idx):
        """Per-step QKV work broken into small PE quanta so it can be
        interleaved into the previous step's attention waves."""
        (b_q, j_q) = steps[idx]
        gg_q = b_q * GPB + j_q
        cols_q = slice(gg_q * 512, (gg_q + 1) * 512)
        thunks = []
        state = {}
        for (w_s, b_s, dst) in ((wq_s, bq_s, qT), (wk_s, bk_s, kT),
                                (wv_s, bv_s, vT)):
            def chain(k0, w_s=w_s, b_s=b_s, dst=dst):
                if k0 == 0:
                    state['acc'] = psA.tile([128, 512], F32, tag="accop",
                                            name="acc")
                acc = state['acc']
                for k in (k0, k0 + 1):
                    nc.tensor.matmul(acc[:], w_s[:, k, :], xgs[idx][:, k, :],
                                     start=(k == 0), stop=(k == KC - 1))
                if k0 == KC - 2:
                    nc.vector.tensor_scalar_add(dst[:, cols_q], acc[:],
                                                b_s[:])
            for k0 in range(0, KC, 2):
                thunks.append(lambda k0=k0, chain=chain: chain(k0))
        for i in range(4):
            def trans(i=i):
                cg = gg_q * 4 + i
                vslice = vT[:, cg * 128:(cg + 1) * 128]
                tp = psA.tile([128, 128], MM_DT, tag="accop")
                nc.tensor.transpose(tp[:], vslice, id_s[:])
                nc.vector.tensor_copy(
                    vaug[:, cg, :, 0:64],
                    tp[:].rearrange("p (h d) -> p h d", h=HPC))
            thunks.append(trans)
        return thunks

    # PE warmup: dummy matmuls on a memset tile so the HAM clock-gate opens
    # (~3.4us of activity) before the first real QKV chain arrives - no
    # data deps, so these start right after the kernel preamble
    warm = const.tile([128, 512], MM_DT, tag="warm")
    nc.vector.memset(warm[:], 0.0)
    wacc = psA.tile([128, 512], F32, tag="accop", name="wacc")
    for k in range(12):
        nc.tensor.matmul(wacc[:], warm[:, 0:128], warm[:],
                         start=(k == 0), stop=(k == 11))

    # prologue: weights first on sync (small, unblock the first LDW), xg0
    # split right behind so the first chain starts on the first half; xg1
    # also on sync (SWDGE delivery measured too late for step 1)
    # exactly 8 prologue DMAs on the sync ring: one per DMA lane, so no
    # prologue tile's completion aliases with a later po-write on its lane
    nc.sync.dma_start(wq_s[:], wq[:])
    nc.sync.dma_start(wk_s[:], wk[:])
    nc.sync.dma_start(wv_s[:], wv[:])
    emit_xg_load(0, nc.scalar)
    emit_xg_load(1, nc.scalar)
    nc.sync.dma_start(bqkv_s[:], bqkv[:])
    nc.sync.dma_start(trid_s[:], trid[:])
    nc.sync.dma_start(wo_s[:], wo[:])
    # ones columns 64:128 of V_aug (softmax denominator, replicated over the
    # 64 PSUM partitions 64:127 so normalize needs no partition broadcast).
    # memset, not DMA: a DMA's completion lands on a shared DMA-lane counter
    # that later po writes keep advancing, so every AV matmul would pick up
    # false waits on po-write completions.
    va_ones = vaug[:].rearrange("p c h x -> p (c h) x")[:, :, 64:128]
    nc.gpsimd.memset(va_ones, 1.0)
    emit_xg_load(2)
    for t in qkv_quanta(0):
        t()
    carry_outproj = []

    for idx, (b, j) in enumerate(steps):
        gg = b * GPB + j
        cols = slice(gg * 512, (gg + 1) * 512)

        # work to interleave into this step's attention waves: the previous
        # step's output projection (carried, so it fills wave slack instead
        # of bursting ahead of ACT), next step's QKV, and an xg prefetch
        from collections import deque as _dq
        quanta = _dq()
        if idx + 3 < len(steps):
            quanta.append(lambda i=idx + 3: emit_xg_load(i))
        quanta.extend(carry_outproj)
        carry_outproj = []
        if idx + 1 < len(steps):
            quanta.extend(qkv_quanta(idx + 1))
        n_waves = 4 * j + 3
        waves_left = [n_waves]   # mutable for the adaptive pop below
        last_step = (idx == len(steps) - 1)

        per_wave = max(1, -(-len(quanta) // n_waves))

        def pop_quanta():
            # front-load (keeps early waves dense for the HAM clock-gate)
            # but reserve one quantum per remaining wave so no wave is bare
            k = min(per_wave, max(1, len(quanta) - (waves_left[0] - 1)))
            for _ in range(k):
                if quanta:
                    quanta.popleft()()
            waves_left[0] -= 1

        # ---- attention blocks unlocked by this group
        for qi in (2 * j, 2 * j + 1):
            qcols = slice(b * S + qi * 256, b * S + qi * 256 + 256)
            nch = 2 * (qi + 1)          # kv chunks of 128 for this q block
            ct0 = psC.tile([128, 256], F32, tag="ct0")
            ct1 = psC.tile([128, 256], F32, tag="ct1")
            cts = [ct0, ct1]
            def av_wave(w, e):
                # AV accumulation for wave w (kv chunks 2w, 2w+1); alternate
                # heads so consecutive matmuls hit different PSUM banks
                for i, jj in enumerate((2 * w, 2 * w + 1)):
                    for h in range(HPC):
                        lhsT = vaug[:, b * (S // 128) + jj, h, :]
                        first = (jj == 0)
                        last = (jj == nch - 1)
                        if jj == nch - 2:  # even diag chunk: mask lower tri
                            nc.gpsimd.tensor_mul(
                                e[:, 2 * h + i, 0:128],
                                e[:, 2 * h + i, 0:128], tri_s[:])
                            nc.tensor.matmul(cts[h][:, :], lhsT,
                                             e[:, 2 * h + i, :],
                                             start=first, stop=last)
                        elif jj == nch - 1:  # odd diag: q first half masked
                            nc.gpsimd.tensor_mul(
                                e[:, 2 * h + i, 128:256],
                                e[:, 2 * h + i, 128:256], tri_s[:])
                            nc.tensor.matmul(cts[h][:, 128:256], lhsT,
                                             e[:, 2 * h + i, 128:256],
                                             start=first, stop=last)
                        else:
                            nc.tensor.matmul(cts[h][:, :], lhsT,
                                             e[:, 2 * h + i, :],
                                             start=first, stop=last)

            # software pipeline: emit scores(w)+exp(w) before AV(w-1), so the
            # PE instruction stream is S0 S1 A0 S2 A1 ... and exp overlaps AV
            if qi == 2 * j:
                # delay PE's arrival at the step's first scores matmul (its
                # PSUM slot frees only when the exp backlog drains ~1us later)
                for _ in range(2):
                    if quanta:
                        quanta.popleft()()
            prev_e = None
            for w in range(qi + 1):
                js = (2 * w, 2 * w + 1)
                sp = psS.tile([128, 2 * HPC, 256], SP_DT, tag="sp")
                for h in range(HPC):
                    for i, jj in enumerate(js):
                        kcols = slice(b * S + jj * 128,
                                      b * S + jj * 128 + 128)
                        nc.tensor.matmul(
                            sp[:, 2 * h + i, :],
                            kT[h * 64:(h + 1) * 64, kcols],
                            qT[h * 64:(h + 1) * 64, qcols],
                            start=True, stop=True)
                e = ep.tile([128, 2 * HPC, 256], MM_DT, tag="e")
                nc.scalar.activation(e[:], sp[:], EXPFN, scale=0.125)
                pop_quanta()
                if prev_e is not None:
                    av_wave(w - 1, prev_e)
                prev_e = e
            av_wave(qi, prev_e)
            # normalize: ctxT[:, q] = ct[0:64] * recip(ct[64:128]) - the
            # denominator is already replicated across partitions 64:127
            for h in range(HPC):
                # tensor_copy is the only DVE op that honors a partition
                # shift; custom-DVE (recip) and TT reads do not, so copy the
                # replicated denominators down to partitions 0:63 first
                d = rp.tile([64, 256], F32, tag="d")
                nc.vector.tensor_copy(d[:], cts[h][64:128, :])
                r = rp.tile([64, 256], F32, tag="r")
                nc.vector.reciprocal_approx_fast(r[:], d[:])
                nc.vector.tensor_mul(ctxT[h * 64:(h + 1) * 64, qcols],
                                     cts[h][0:64, :], r[:])
                if t_out_dbg is not None:
                    nc.vector.tensor_copy(
                        dbg_craw_s[h * 64:(h + 1) * 64, qcols],
                        cts[h][0:64, :])
                    nc.vector.tensor_copy(
                        dbg_den_s[h * 64:(h + 1) * 64, qcols],
                        cts[h][64:128, :])

            if last_step and qi == 2 * j:
                # feed the final block's waves with the first half's output
                # projection (no next-step QKV left to fill PE slack)
                half = slice(gg * 512, gg * 512 + 256)
                osth = osp.tile([128, KC, 256], MM_DT, tag="ost",
                                name="osth")
                for c in range(KC):
                    def proj_half(c=c):
                        op = psA.tile([128, 256], SP_DT, tag="accop",
                                      name="oph")
                        nc.tensor.matmul(op[:], woc[:, c, :], ctxT[:, half],
                                         start=True, stop=True)
                        nc.vector.tensor_copy(osth[:, c, :], op[:])
                        if c == KC - 1:
                            nc.sync.dma_start(po_pview[:, :, half], osth[:])
                    quanta.append(proj_half)

        while quanta:          # leftover next-step QKV work
            quanta.popleft()()

        # ---- output projection for the completed 512-token group
        # (the last step projected its first half inside the final block's
        # waves - see proj_half - so only the second half remains).
        # All 8 chunks stage into one SBUF tile and ship as a single DMA -
        # fewer DMA-lane updates means fewer conservative cross-waits.
        # Non-final steps defer the chunks into the next step's waves.
        pcols = slice(gg * 512 + 256, (gg + 1) * 512) if last_step else cols
        pn = 256 if last_step else 512
        ostb = osp.tile([128, KC, pn], MM_DT, tag="ost", name="ostb")

        def proj_chunk(c, pcols=pcols, pn=pn, ostb=ostb):
            op = psA.tile([128, pn], SP_DT, tag="accop", name="op")
            nc.tensor.matmul(op[:], woc[:, c, :], ctxT[:, pcols],
                             start=True, stop=True)
            nc.vector.tensor_copy(ostb[:, c, :], op[:])
            if c == KC - 1:
                nc.sync.dma_start(po_pview[:, :, pcols], ostb[:])

        if last_step:
            for c in range(KC):
                proj_chunk(c)
        else:
            carry_outproj = [lambda c=c: proj_chunk(c) for c in range(KC)]

    if t_out_dbg is not None:
        dq, dk, dc, dv, dcr, dde = t_out_dbg
        st = ctx.enter_context(tc.tile_pool(name="dbg", bufs=1))
        for src, dst in ((qT, dq), (kT, dk), (ctxT, dc)):
            tmp = st.tile([128, T], F32, tag="dbgt")
            nc.vector.tensor_copy(tmp[:], src[:])
            nc.sync.dma_start(dst[:], tmp[:])
        nc.sync.dma_start(dcr[:], dbg_craw_s[:])
        nc.sync.dma_start(dde[:], dbg_den_s[:])
        tmpv = st.tile([128, NKV * HPC * 128], F32, tag="dbgt")
        nc.vector.tensor_copy(
            tmpv[:], vaug[:].rearrange("p c h x -> p (c h x)"))
        nc.sync.dma_start(dv[:], tmpv[:])


_NC = None


def _build():
    global _NC
    if _NC is not None:
        return _NC
    nc = bacc.Bacc("TRN2", target_bir_lowering=False, debug=False,
                   num_devices=NCORES)
    t_in = [
        nc.dram_tensor("xt", [NG, 128, KC, 512], MM_DT, kind="ExternalInput").ap(),
        nc.dram_tensor("wq", [128, KC, 128], MM_DT, kind="ExternalInput").ap(),
        nc.dram_tensor("wk", [128, KC, 128], MM_DT, kind="ExternalInput").ap(),
        nc.dram_tensor("wv", [128, KC, 128], MM_DT, kind="ExternalInput").ap(),
        nc.dram_tensor("wo", [128, D], MM_DT, kind="ExternalInput").ap(),
        nc.dram_tensor("bqkv", [128, 3], F32, kind="ExternalInput").ap(),
        nc.dram_tensor("trid", [128, 2, 128], MM_DT, kind="ExternalInput").ap(),
    ]
    po = nc.dram_tensor("po", [KC, 128, T], MM_DT, kind="ExternalOutput").ap()
    t_out_dbg = None
    if os.environ.get("KERNEL_DEBUG_TAPS") == "1":
        t_out_dbg = [
            nc.dram_tensor("dbg_qT", [128, T], F32, kind="ExternalOutput").ap(),
            nc.dram_tensor("dbg_kT", [128, T], F32, kind="ExternalOutput").ap(),
            nc.dram_tensor("dbg_ctxT", [128, T], F32, kind="ExternalOutput").ap(),
            nc.dram_tensor("dbg_vaug", [128, NKV * HPC * 128], F32,
                           kind="ExternalOutput").ap(),
            nc.dram_tensor("dbg_craw", [128, T], F32, kind="ExternalOutput").ap(),
            nc.dram_tensor("dbg_den", [128, T], F32, kind="ExternalOutput").ap(),
        ]
    with tile.TileContext(nc) as tc, ExitStack() as ctx:
        _body(nc, tc, ctx, t_in, po, t_out_dbg)
    nc.compile()
    _NC = nc
    return nc


def _in_maps(hidden_states, Wq, bq, Wk, bk, Wv, bv, Wo, bo):
    hid = np.asarray(hidden_states, dtype=np.float32).reshape(T, D)
    hidT = hid.T.astype(MM_NP)                       # [D, T]
    xt = np.ascontiguousarray(
        hidT.reshape(KC, 128, NG, 512).transpose(2, 1, 0, 3))
    common = {
        "xt": xt,
        "trid": np.ascontiguousarray(np.stack(
            [np.triu(np.ones((128, 128), MM_NP)),
             np.eye(128, dtype=MM_NP)], axis=1)),
    }
    maps = []
    for c in range(NCORES):
        cs = slice(c * 128, (c + 1) * 128)
        maps.append(dict(
            common,
            wq=np.ascontiguousarray(np.asarray(Wq)[:, cs].astype(MM_NP).reshape(KC, 128, 128).transpose(1, 0, 2)),
            wk=np.ascontiguousarray(np.asarray(Wk)[:, cs].astype(MM_NP).reshape(KC, 128, 128).transpose(1, 0, 2)),
            wv=np.ascontiguousarray(np.asarray(Wv)[:, cs].astype(MM_NP).reshape(KC, 128, 128).transpose(1, 0, 2)),
            wo=np.ascontiguousarray(np.asarray(Wo)[cs, :].astype(MM_NP)),
            bqkv=np.ascontiguousarray(np.stack(
                [np.asarray(bq)[cs], np.asarray(bk)[cs],
                 np.asarray(bv)[cs]], axis=1).astype(np.float32)),
        ))
    return maps


def kernel(hidden_states, Wq, bq, Wk, bk, Wv, bv, Wo, bo):
    nc = _build()
    maps = _in_maps(hidden_states, Wq, bq, Wk, bk, Wv, bv, Wo, bo)
    res = run_bass_kernel_spmd(nc, maps, list(range(NCORES))).results
    acc = np.zeros((KC, 128, T), np.float32)
    for r in res:
        acc += r["po"].astype(np.float32)
    outT = acc.reshape(D, T)
    out = outT.T + np.asarray(bo, dtype=np.float32)[None, :]
    return out.reshape(B, S, D).astype(np.float32)


# revision 62
# speedup vs baseline: 1.0232x; 1.0232x over previous
"""Multi-head causal attention (B=2, S=2048, H=16, DH=64, D=1024) on 8 TRN2 cores.

Sharding: Megatron tensor-parallel over heads - core c owns heads {2c, 2c+1}:
  * column-slices of Wq/Wk/Wv (128 cols each) + bias slices,
  * row-slice of Wo (128 rows),
  * full hidden_states (pre-transposed on host to [D, B*S]).
Each core computes a partial output (its 2 heads through Wo rows); host sums
the 8 f16 partials (row-parallel unshard) and adds bo.

Device dataflow per core, interleaved so PE (matmul), ACT (exp) and DVE
(evacuations) overlap across phases; 512-token groups alternate batches so
attention blocks unlock progressively:
  per group g (b, j):
  A) QKV: qT/kT/vT [128, 512] = W_slice.T @ hiddenT chunk (contraction over D
     in 8 chunks of 128, f32 PSUM accumulate; bias added on evacuation).
     vT columns transposed into V_aug [tok128, chunk, head, 128] via DMA-xbar
     transpose; V_aug cols 64:128 are pre-set to ones so AV matmuls emit the
     softmax denominator replicated on PSUM partitions 64:127.
  B) attention blocks (b, 2j) and (b, 2j+1), both heads, causal:
     scoresT[kv,q] = kT_slice.T @ qT_slice (K=64, N=256, the two heads run
     concurrently on PE row-groups 0-63/64-127), exp via ACT straight off
     f16 PSUM, diagonal masked by triangular multiplies on GPSIMD, then
     ct[128, q] += V_aug.T @ expT (rows 0:64 ctx, 64:128 denominator).
     Normalize: DVE reciprocal of rows 64:128, multiply into ctxT.
  C) output projection for the finished 512 tokens: f16 PSUM matmul, DVE
     copy to SBUF, DMA to DRAM in f16.
"""
import os
import sys

sys.path.insert(0, "/opt/trn_rl_repo")

from contextlib import ExitStack

import numpy as np

import concourse.bass as bass
import concourse.mybir as mybir
import concourse.tile as tile
from concourse import bacc
from concourse.bass_utils import run_bass_kernel_spmd

F32 = mybir.dt.float32
F16 = mybir.dt.float16
MM_DT = F16
MM_NP = np.float16

B, S, H, DH = 2, 2048, 16, 64
D = H * DH            # 1024
T = B * S             # 4096 tokens
NCORES = 8
HPC = H // NCORES     # 2 heads per core
KC = D // 128         # 8 contraction chunks
NG = T // 512         # 8 token groups
GPB = NG // B         # 4 groups per batch
NKV = T // 128        # 32 kv chunks of 128 tokens
EXPFN = mybir.ActivationFunctionType.Exp

# DMA-xbar transpose serializes the whole HWDGE ring under Tile's
# hang-workaround (measured ~1.2us each + completion waits) - PE default.
USE_DMAT = os.environ.get("KERNEL_DMAT") == "1"
SP_DT = F32  # matmul outputs must be fp32 in PSUM


def _body(nc, tc, ctx, t_in, t_out, t_out_dbg=None):
    xt, wq, wk, wv, wo, bqkv, trid = t_in
    po = t_out

    const = ctx.enter_context(tc.tile_pool(name="const", bufs=1))
    big = ctx.enter_context(tc.tile_pool(name="big", bufs=1))
    xtp = ctx.enter_context(tc.tile_pool(name="xtp", bufs=4))
    ep = ctx.enter_context(tc.tile_pool(name="ep", bufs=8))
    rp = ctx.enter_context(tc.tile_pool(name="rp", bufs=4))
    osp = ctx.enter_context(tc.tile_pool(name="osp", bufs=4))
    vsp = ctx.enter_context(tc.tile_pool(name="vsp", bufs=3))

    # PSUM budget (8 banks):
    #   sp f32 2 banks x2 + ct0/ct1 1 bank x1 each + accop 1 bank x2 = 8
    ct_bufs = 1
    psS = ctx.enter_context(tc.tile_pool(name="psS", bufs=2, space="PSUM"))
    psC = ctx.enter_context(tc.tile_pool(name="psC", bufs=ct_bufs, space="PSUM"))
    psA = ctx.enter_context(tc.tile_pool(name="psA", bufs=2, space="PSUM"))

    # ---- constants / weights in SBUF
    wq_s = const.tile([128, KC, 128], MM_DT, tag="wq")
    wk_s = const.tile([128, KC, 128], MM_DT, tag="wk")
    wv_s = const.tile([128, KC, 128], MM_DT, tag="wv")
    wo_s = const.tile([128, D], MM_DT, tag="wo")
    bqkv_s = const.tile([128, 3], F32, tag="bqkv")
    trid_s = const.tile([128, 2, 128], MM_DT, tag="trid")
    bq_s, bk_s, bv_s = (bqkv_s[:, 0:1], bqkv_s[:, 1:2], bqkv_s[:, 2:3])
    tri_s, id_s = trid_s[:, 0, :], trid_s[:, 1, :]
    qT = big.tile([128, T], MM_DT, tag="qT")
    kT = big.tile([128, T], MM_DT, tag="kT")
    vT = big.tile([128, T], MM_DT, tag="vT")
    ctxT = big.tile([128, T], MM_DT, tag="ctxT")
    vaug = big.tile([128, NKV, HPC, 128], MM_DT, tag="vaug")

    woc = wo_s[:].rearrange("p (c n) -> p c n", c=KC)
    # partition-major view of po for single-DMA stores: [128, KC, T]
    po_pview = po.rearrange("c p t -> p c t")

    if t_out_dbg is not None:
        dbgp = ctx.enter_context(tc.tile_pool(name="dbgp", bufs=1))
        dbg_craw_s = dbgp.tile([128, T], F32, tag="craw")
        dbg_den_s = dbgp.tile([128, T], F32, tag="den")

    # groups alternate batches so attention work unlocks early and evenly
    steps = [(b, j) for j in range(GPB) for b in range(B)]

    xgs = {}

    def emit_xg_load(idx, engine=None, split=False):
        # prefetch the hidden-state slice for step idx (gpsimd/SWDGE ring so
        # it is not queued behind the po stores on the sync ring); split=True
        # halves the transfer so the first chain can start on the first half
        gg_l = steps[idx][0] * GPB + steps[idx][1]
        xg = xtp.tile([128, KC, 512], MM_DT, tag="xt")
        eng = engine or nc.gpsimd
        if split:
            eng.dma_start(xg[:, 0:KC // 2, :], xt[gg_l][:, 0:KC // 2, :])
            eng.dma_start(xg[:, KC // 2:, :], xt[gg_l][:, KC // 2:, :])
        else:
            eng.dma_start(xg[:], xt[gg_l])
        xgs[idx] = xg

    def qkv_quanta(idx):
        """Per-step QKV work broken into small PE quanta so it can be
        interleaved into the previous step's attention waves."""
        (b_q, j_q) = steps[idx]
        gg_q = b_q * GPB + j_q
        cols_q = slice(gg_q * 512, (gg_q + 1) * 512)
        thunks = []
        state = {}
        for (w_s, b_s, dst) in ((wq_s, bq_s, qT), (wk_s, bk_s, kT),
                                (wv_s, bv_s, vT)):
            def chain(k0, w_s=w_s, b_s=b_s, dst=dst):
                if k0 == 0:
                    state['acc'] = psA.tile([128, 512], F32, tag="accop",
                                            name="acc")
                acc = state['acc']
                for k in (k0, k0 + 1):
                    nc.tensor.matmul(acc[:], w_s[:, k, :], xgs[idx][:, k, :],
                                     start=(k == 0), stop=(k == KC - 1))
                if k0 == KC - 2:
                    nc.vector.tensor_scalar_add(dst[:, cols_q], acc[:],
                                                b_s[:])
            for k0 in range(0, KC, 2):
                thunks.append(lambda k0=k0, chain=chain: chain(k0))
        for i in range(4):
            def trans(i=i):
                cg = gg_q * 4 + i
                vslice = vT[:, cg * 128:(cg + 1) * 128]
                tp = psA.tile([128, 128], MM_DT, tag="accop")
                nc.tensor.transpose(tp[:], vslice, id_s[:])
                nc.vector.tensor_copy(
                    vaug[:, cg, :, 0:64],
                    tp[:].rearrange("p (h d) -> p h d", h=HPC))
            thunks.append(trans)
        return thunks

    # PE warmup: dummy matmuls on a memset tile so the HAM clock-gate opens
    # (~3.4us of activity) before the first real QKV chain arrives - no
    # data deps, so these start right after the kernel preamble
    warm = const.tile([128, 512], MM_DT, tag="warm")
    nc.vector.memset(warm[:], 0.0)
    wacc = psA.tile([128, 512], F32, tag="accop", name="wacc")
    for k in range(12):
        nc.tensor.matmul(wacc[:], warm[:, 0:128], warm[:],
                         start=(k == 0), stop=(k == 11))

    # prologue: weights first on sync (small, unblock the first LDW), xg0
    # split right behind so the first chain starts on the first half; xg1
    # also on sync (SWDGE delivery measured too late for step 1)
    # exactly 8 prologue DMAs on the sync ring: one per DMA lane, so no
    # prologue tile's completion aliases with a later po-write on its lane
    nc.sync.dma_start(wq_s[:], wq[:])
    nc.sync.dma_start(wk_s[:], wk[:])
    nc.sync.dma_start(wv_s[:], wv[:])
    emit_xg_load(0, nc.scalar)
    emit_xg_load(1, nc.scalar)
    nc.sync.dma_start(bqkv_s[:], bqkv[:])
    nc.sync.dma_start(trid_s[:], trid[:])
    nc.sync.dma_start(wo_s[:], wo[:])
    # ones columns 64:128 of V_aug (softmax denominator, replicated over the
    # 64 PSUM partitions 64:127 so normalize needs no partition broadcast).
    # memset, not DMA: a DMA's completion lands on a shared DMA-lane counter
    # that later po writes keep advancing, so every AV matmul would pick up
    # false waits on po-write completions.
    va_ones = vaug[:].rearrange("p c h x -> p (c h) x")[:, :, 64:128]
    nc.gpsimd.memset(va_ones, 1.0)
    emit_xg_load(2)
    for t in qkv_quanta(0):
        t()
    carry_outproj = []

    for idx, (b, j) in enumerate(steps):
        gg = b * GPB + j
        cols = slice(gg * 512, (gg + 1) * 512)

        # work to interleave into this step's attention waves: the previous
        # step's output projection (carried, so it fills wave slack instead
        # of bursting ahead of ACT), next step's QKV, and an xg prefetch
        from collections import deque as _dq
        quanta = _dq()
        if idx + 3 < len(steps):
            quanta.append(lambda i=idx + 3: emit_xg_load(i))
        quanta.extend(carry_outproj)
        carry_outproj = []
        if idx + 1 < len(steps):
            quanta.extend(qkv_quanta(idx + 1))
        n_waves = 4 * j + 3
        waves_left = [n_waves]   # mutable for the adaptive pop below
        last_step = (idx == len(steps) - 1)

        per_wave = max(1, -(-len(quanta) // n_waves))

        def pop_quanta():
            # front-load (keeps early waves dense for the HAM clock-gate)
            # but reserve one quantum per remaining wave so no wave is bare
            k = min(per_wave, max(1, len(quanta) - (waves_left[0] - 1)))
            for _ in range(k):
                if quanta:
                    quanta.popleft()()
            waves_left[0] -= 1

        # ---- attention blocks unlocked by this group
        for qi in (2 * j, 2 * j + 1):
            qcols = slice(b * S + qi * 256, b * S + qi * 256 + 256)
            nch = 2 * (qi + 1)          # kv chunks of 128 for this q block
            ct0 = psC.tile([128, 256], F32, tag="ct0")
            ct1 = psC.tile([128, 256], F32, tag="ct1")
            cts = [ct0, ct1]
            def av_wave(w, e):
                # AV accumulation for wave w (kv chunks 2w, 2w+1); alternate
                # heads so consecutive matmuls hit different PSUM banks
                for i, jj in enumerate((2 * w, 2 * w + 1)):
                    for h in range(HPC):
                        lhsT = vaug[:, b * (S // 128) + jj, h, :]
                        first = (jj == 0)
                        last = (jj == nch - 1)
                        if jj == nch - 2:  # even diag chunk: mask lower tri
                            nc.gpsimd.tensor_mul(
                                e[:, 2 * h + i, 0:128],
                                e[:, 2 * h + i, 0:128], tri_s[:])
                            nc.tensor.matmul(cts[h][:, :], lhsT,
                                             e[:, 2 * h + i, :],
                                             start=first, stop=last)
                        elif jj == nch - 1:  # odd diag: q first half masked
                            nc.gpsimd.tensor_mul(
                                e[:, 2 * h + i, 128:256],
                                e[:, 2 * h + i, 128:256], tri_s[:])
                            nc.tensor.matmul(cts[h][:, 128:256], lhsT,
                                             e[:, 2 * h + i, 128:256],
                                             start=first, stop=last)
                        else:
                            nc.tensor.matmul(cts[h][:, :], lhsT,
                                             e[:, 2 * h + i, :],
                                             start=first, stop=last)

            # software pipeline: emit scores(w)+exp(w) before AV(w-1), so the
            # PE instruction stream is S0 S1 A0 S2 A1 ... and exp overlaps AV
            prev_e = None
            for w in range(qi + 1):
                js = (2 * w, 2 * w + 1)
                sp = psS.tile([128, 2 * HPC, 256], SP_DT, tag="sp")
                for h in range(HPC):
                    for i, jj in enumerate(js):
                        kcols = slice(b * S + jj * 128,
                                      b * S + jj * 128 + 128)
                        nc.tensor.matmul(
                            sp[:, 2 * h + i, :],
                            kT[h * 64:(h + 1) * 64, kcols],
                            qT[h * 64:(h + 1) * 64, qcols],
                            start=True, stop=True)
                e = ep.tile([128, 2 * HPC, 256], MM_DT, tag="e")
                nc.scalar.activation(e[:], sp[:], EXPFN, scale=0.125)
                pop_quanta()
                if prev_e is not None:
                    av_wave(w - 1, prev_e)
                prev_e = e
            av_wave(qi, prev_e)
            # normalize: ctxT[:, q] = ct[0:64] * recip(ct[64:128]) - the
            # denominator is already replicated across partitions 64:127
            for h in range(HPC):
                # tensor_copy is the only DVE op that honors a partition
                # shift; custom-DVE (recip) and TT reads do not, so copy the
                # replicated denominators down to partitions 0:63 first
                d = rp.tile([64, 256], F32, tag="d")
                nc.vector.tensor_copy(d[:], cts[h][64:128, :])
                r = rp.tile([64, 256], F32, tag="r")
                nc.vector.reciprocal_approx_fast(r[:], d[:])
                nc.vector.tensor_mul(ctxT[h * 64:(h + 1) * 64, qcols],
                                     cts[h][0:64, :], r[:])
                if t_out_dbg is not None:
                    nc.vector.tensor_copy(
                        dbg_craw_s[h * 64:(h + 1) * 64, qcols],
                        cts[h][0:64, :])
                    nc.vector.tensor_copy(
                        dbg_den_s[h * 64:(h + 1) * 64, qcols],
                        cts[h][64:128, :])

            if last_step and qi == 2 * j:
                # feed the final block's waves with the first half's output
                # projection (no next-step QKV left to fill PE slack)
                half = slice(gg * 512, gg * 512 + 256)
                osth = osp.tile([128, KC, 256], MM_DT, tag="ost",
                                name="osth")
                for c in range(KC):
                    def proj_half(c=c):
                        op = psA.tile([128, 256], SP_DT, tag="accop",
                                      name="oph")
                        nc.tensor.matmul(op[:], woc[:, c, :], ctxT[:, half],
                                         start=True, stop=True)
                        nc.vector.tensor_copy(osth[:, c, :], op[:])
                        if c == KC - 1:
                            nc.sync.dma_start(po_pview[:, :, half], osth[:])
                    quanta.append(proj_half)

        while quanta:          # leftover next-step QKV work
            quanta.popleft()()

        # ---- output projection for the completed 512-token group
        # (the last step projected its first half inside the final block's
        # waves - see proj_half - so only the second half remains).
        # All 8 chunks stage into one SBUF tile and ship as a single DMA -
        # fewer DMA-lane updates means fewer conservative cross-waits.
        # Non-final steps defer the chunks into the next step's waves.
        pcols = slice(gg * 512 + 256, (gg + 1) * 512) if last_step else cols
        pn = 256 if last_step else 512
        ostb = osp.tile([128, KC, pn], MM_DT, tag="ost", name="ostb")

        def proj_chunk(c, pcols=pcols, pn=pn, ostb=ostb):
            op = psA.tile([128, pn], SP_DT, tag="accop", name="op")
            nc.tensor.matmul(op[:], woc[:, c, :], ctxT[:, pcols],
                             start=True, stop=True)
            nc.vector.tensor_copy(ostb[:, c, :], op[:])
            if c == KC - 1:
                nc.sync.dma_start(po_pview[:, :, pcols], ostb[:])

        if last_step:
            for c in range(KC):
                proj_chunk(c)
        else:
            carry_outproj = [lambda c=c: proj_chunk(c) for c in range(KC)]

    if t_out_dbg is not None:
        dq, dk, dc, dv, dcr, dde = t_out_dbg
        st = ctx.enter_context(tc.tile_pool(name="dbg", bufs=1))
        for src, dst in ((qT, dq), (kT, dk), (ctxT, dc)):
            tmp = st.tile([128, T], F32, tag="dbgt")
            nc.vector.tensor_copy(tmp[:], src[:])
            nc.sync.dma_start(dst[:], tmp[:])
        nc.sync.dma_start(dcr[:], dbg_craw_s[:])
        nc.sync.dma_start(dde[:], dbg_den_s[:])
        tmpv = st.tile([128, NKV * HPC * 128], F32, tag="dbgt")
        nc.vector.tensor_copy(
            tmpv[:], vaug[:].rearrange("p c h x -> p (c h x)"))
        nc.sync.dma_start(dv[:], tmpv[:])


_NC = None


def _build():
    global _NC
    if _NC is not None:
        return _NC
    nc = bacc.Bacc("TRN2", target_bir_lowering=False, debug=False,
                   num_devices=NCORES)
    t_in = [
        nc.dram_tensor("xt", [NG, 128, KC, 512], MM_DT, kind="ExternalInput").ap(),
        nc.dram_tensor("wq", [128, KC, 128], MM_DT, kind="ExternalInput").ap(),
        nc.dram_tensor("wk", [128, KC, 128], MM_DT, kind="ExternalInput").ap(),
        nc.dram_tensor("wv", [128, KC, 128], MM_DT, kind="ExternalInput").ap(),
        nc.dram_tensor("wo", [128, D], MM_DT, kind="ExternalInput").ap(),
        nc.dram_tensor("bqkv", [128, 3], F32, kind="ExternalInput").ap(),
        nc.dram_tensor("trid", [128, 2, 128], MM_DT, kind="ExternalInput").ap(),
    ]
    po = nc.dram_tensor("po", [KC, 128, T], MM_DT, kind="ExternalOutput").ap()
    t_out_dbg = None
    if os.environ.get("KERNEL_DEBUG_TAPS") == "1":
        t_out_dbg = [
            nc.dram_tensor("dbg_qT", [128, T], F32, kind="ExternalOutput").ap(),
            nc.dram_tensor("dbg_kT", [128, T], F32, kind="ExternalOutput").ap(),
            nc.dram_tensor("dbg_ctxT", [128, T], F32, kind="ExternalOutput").ap(),
            nc.dram_tensor("dbg_vaug", [128, NKV * HPC * 128], F32,
                           kind="ExternalOutput").ap(),
            nc.dram_tensor("dbg_craw", [128, T], F32, kind="ExternalOutput").ap(),
            nc.dram_tensor("dbg_den", [128, T], F32, kind="ExternalOutput").ap(),
        ]
    with tile.TileContext(nc) as tc, ExitStack() as ctx:
        _body(nc, tc, ctx, t_in, po, t_out_dbg)
    nc.compile()
    _NC = nc
    return nc


def _in_maps(hidden_states, Wq, bq, Wk, bk, Wv, bv, Wo, bo):
    hid = np.asarray(hidden_states, dtype=np.float32).reshape(T, D)
    hidT = hid.T.astype(MM_NP)                       # [D, T]
    xt = np.ascontiguousarray(
        hidT.reshape(KC, 128, NG, 512).transpose(2, 1, 0, 3))
    common = {
        "xt": xt,
        "trid": np.ascontiguousarray(np.stack(
            [np.triu(np.ones((128, 128), MM_NP)),
             np.eye(128, dtype=MM_NP)], axis=1)),
    }
    maps = []
    for c in range(NCORES):
        cs = slice(c * 128, (c + 1) * 128)
        maps.append(dict(
            common,
            wq=np.ascontiguousarray(np.asarray(Wq)[:, cs].astype(MM_NP).reshape(KC, 128, 128).transpose(1, 0, 2)),
            wk=np.ascontiguousarray(np.asarray(Wk)[:, cs].astype(MM_NP).reshape(KC, 128, 128).transpose(1, 0, 2)),
            wv=np.ascontiguousarray(np.asarray(Wv)[:, cs].astype(MM_NP).reshape(KC, 128, 128).transpose(1, 0, 2)),
            wo=np.ascontiguousarray(np.asarray(Wo)[cs, :].astype(MM_NP)),
            bqkv=np.ascontiguousarray(np.stack(
                [np.asarray(bq)[cs], np.asarray(bk)[cs],
                 np.asarray(bv)[cs]], axis=1).astype(np.float32)),
        ))
    return maps


def kernel(hidden_states, Wq, bq, Wk, bk, Wv, bv, Wo, bo):
    nc = _build()
    maps = _in_maps(hidden_states, Wq, bq, Wk, bk, Wv, bv, Wo, bo)
    res = run_bass_kernel_spmd(nc, maps, list(range(NCORES))).results
    acc = np.zeros((KC, 128, T), np.float32)
    for r in res:
        acc += r["po"].astype(np.float32)
    outT = acc.reshape(D, T)
    out = outT.T + np.asarray(bo, dtype=np.float32)[None, :]
    return out.reshape(B, S, D).astype(np.float32)


# revision 63
# speedup vs baseline: 1.0400x; 1.0164x over previous
"""Multi-head causal attention (B=2, S=2048, H=16, DH=64, D=1024) on 8 TRN2 cores.

Sharding: Megatron tensor-parallel over heads - core c owns heads {2c, 2c+1}:
  * column-slices of Wq/Wk/Wv (128 cols each) + bias slices,
  * row-slice of Wo (128 rows),
  * full hidden_states (pre-transposed on host to [D, B*S]).
Each core computes a partial output (its 2 heads through Wo rows); host sums
the 8 f16 partials (row-parallel unshard) and adds bo.

Device dataflow per core, interleaved so PE (matmul), ACT (exp) and DVE
(evacuations) overlap across phases; 512-token groups alternate batches so
attention blocks unlock progressively:
  per group g (b, j):
  A) QKV: qT/kT/vT [128, 512] = W_slice.T @ hiddenT chunk (contraction over D
     in 8 chunks of 128, f32 PSUM accumulate; bias added on evacuation).
     vT columns transposed into V_aug [tok128, chunk, head, 128] via DMA-xbar
     transpose; V_aug cols 64:128 are pre-set to ones so AV matmuls emit the
     softmax denominator replicated on PSUM partitions 64:127.
  B) attention blocks (b, 2j) and (b, 2j+1), both heads, causal:
     scoresT[kv,q] = kT_slice.T @ qT_slice (K=64, N=256, the two heads run
     concurrently on PE row-groups 0-63/64-127), exp via ACT straight off
     f16 PSUM, diagonal masked by triangular multiplies on GPSIMD, then
     ct[128, q] += V_aug.T @ expT (rows 0:64 ctx, 64:128 denominator).
     Normalize: DVE reciprocal of rows 64:128, multiply into ctxT.
  C) output projection for the finished 512 tokens: f16 PSUM matmul, DVE
     copy to SBUF, DMA to DRAM in f16.
"""
import os
import sys

sys.path.insert(0, "/opt/trn_rl_repo")

from contextlib import ExitStack

import numpy as np

import concourse.bass as bass
import concourse.mybir as mybir
import concourse.tile as tile
from concourse import bacc
from concourse.bass_utils import run_bass_kernel_spmd

F32 = mybir.dt.float32
F16 = mybir.dt.float16
MM_DT = F16
MM_NP = np.float16

B, S, H, DH = 2, 2048, 16, 64
D = H * DH            # 1024
T = B * S             # 4096 tokens
NCORES = 8
HPC = H // NCORES     # 2 heads per core
KC = D // 128         # 8 contraction chunks
NG = T // 512         # 8 token groups
GPB = NG // B         # 4 groups per batch
NKV = T // 128        # 32 kv chunks of 128 tokens
EXPFN = mybir.ActivationFunctionType.Exp

# DMA-xbar transpose serializes the whole HWDGE ring under Tile's
# hang-workaround (measured ~1.2us each + completion waits) - PE default.
USE_DMAT = os.environ.get("KERNEL_DMAT") == "1"
SP_DT = F32  # matmul outputs must be fp32 in PSUM


def _body(nc, tc, ctx, t_in, t_out, t_out_dbg=None):
    xt, wq, wk, wv, wo, bqkv, trid = t_in
    po = t_out

    const = ctx.enter_context(tc.tile_pool(name="const", bufs=1))
    big = ctx.enter_context(tc.tile_pool(name="big", bufs=1))
    xtp = ctx.enter_context(tc.tile_pool(name="xtp", bufs=4))
    ep = ctx.enter_context(tc.tile_pool(name="ep", bufs=8))
    rp = ctx.enter_context(tc.tile_pool(name="rp", bufs=4))
    osp = ctx.enter_context(tc.tile_pool(name="osp", bufs=4))
    vsp = ctx.enter_context(tc.tile_pool(name="vsp", bufs=3))

    # PSUM budget (8 banks):
    #   sp f32 2 banks x2 + ct0/ct1 1 bank x1 each + accop 1 bank x2 = 8
    ct_bufs = 1
    psS = ctx.enter_context(tc.tile_pool(name="psS", bufs=2, space="PSUM"))
    psC = ctx.enter_context(tc.tile_pool(name="psC", bufs=ct_bufs, space="PSUM"))
    psA = ctx.enter_context(tc.tile_pool(name="psA", bufs=2, space="PSUM"))

    # ---- constants / weights in SBUF
    wq_s = const.tile([128, KC, 128], MM_DT, tag="wq")
    wk_s = const.tile([128, KC, 128], MM_DT, tag="wk")
    wv_s = const.tile([128, KC, 128], MM_DT, tag="wv")
    wo_s = const.tile([128, D], MM_DT, tag="wo")
    bqkv_s = const.tile([128, 3], F32, tag="bqkv")
    trid_s = const.tile([128, 2, 128], MM_DT, tag="trid")
    bq_s, bk_s, bv_s = (bqkv_s[:, 0:1], bqkv_s[:, 1:2], bqkv_s[:, 2:3])
    tri_s, id_s = trid_s[:, 0, :], trid_s[:, 1, :]
    qT = big.tile([128, T], MM_DT, tag="qT")
    kT = big.tile([128, T], MM_DT, tag="kT")
    vT = big.tile([128, T], MM_DT, tag="vT")
    ctxT = big.tile([128, T], MM_DT, tag="ctxT")
    vaug = big.tile([128, NKV, HPC, 128], MM_DT, tag="vaug")

    woc = wo_s[:].rearrange("p (c n) -> p c n", c=KC)
    # partition-major view of po for single-DMA stores: [128, KC, T]
    po_pview = po.rearrange("c p t -> p c t")

    if t_out_dbg is not None:
        dbgp = ctx.enter_context(tc.tile_pool(name="dbgp", bufs=1))
        dbg_craw_s = dbgp.tile([128, T], F32, tag="craw")
        dbg_den_s = dbgp.tile([128, T], F32, tag="den")

    # groups alternate batches so attention work unlocks early and evenly
    steps = [(b, j) for j in range(GPB) for b in range(B)]

    xgs = {}

    def emit_xg_load(idx, engine=None, split=False):
        # prefetch the hidden-state slice for step idx (gpsimd/SWDGE ring so
        # it is not queued behind the po stores on the sync ring); split=True
        # halves the transfer so the first chain can start on the first half
        gg_l = steps[idx][0] * GPB + steps[idx][1]
        xg = xtp.tile([128, KC, 512], MM_DT, tag="xt")
        eng = engine or nc.gpsimd
        if split:
            eng.dma_start(xg[:, 0:KC // 2, :], xt[gg_l][:, 0:KC // 2, :])
            eng.dma_start(xg[:, KC // 2:, :], xt[gg_l][:, KC // 2:, :])
        else:
            eng.dma_start(xg[:], xt[gg_l])
        xgs[idx] = xg

    def qkv_quanta(idx):
        """Per-step QKV work broken into small PE quanta so it can be
        interleaved into the previous step's attention waves."""
        (b_q, j_q) = steps[idx]
        gg_q = b_q * GPB + j_q
        cols_q = slice(gg_q * 512, (gg_q + 1) * 512)
        thunks = []
        state = {}
        for (w_s, b_s, dst) in ((wq_s, bq_s, qT), (wk_s, bk_s, kT),
                                (wv_s, bv_s, vT)):
            def chain(k0, w_s=w_s, b_s=b_s, dst=dst):
                if k0 == 0:
                    state['acc'] = psA.tile([128, 512], F32, tag="accop",
                                            name="acc")
                acc = state['acc']
                for k in (k0, k0 + 1):
                    nc.tensor.matmul(acc[:], w_s[:, k, :], xgs[idx][:, k, :],
                                     start=(k == 0), stop=(k == KC - 1))
                if k0 == KC - 2:
                    nc.vector.tensor_scalar_add(dst[:, cols_q], acc[:],
                                                b_s[:])
            for k0 in range(0, KC, 2):
                thunks.append(lambda k0=k0, chain=chain: chain(k0))
        for i in range(4):
            def trans(i=i):
                cg = gg_q * 4 + i
                vslice = vT[:, cg * 128:(cg + 1) * 128]
                tp = psA.tile([128, 128], MM_DT, tag="accop")
                nc.tensor.transpose(tp[:], vslice, id_s[:])
                nc.vector.tensor_copy(
                    vaug[:, cg, :, 0:64],
                    tp[:].rearrange("p (h d) -> p h d", h=HPC))
            thunks.append(trans)
        return thunks

    # PE warmup: dummy matmuls on a memset tile so the HAM clock-gate opens
    # (~3.4us of activity) before the first real QKV chain arrives - no
    # data deps, so these start right after the kernel preamble
    warm = const.tile([128, 512], MM_DT, tag="warm")
    nc.vector.memset(warm[:], 0.0)
    wacc = psA.tile([128, 512], F32, tag="accop", name="wacc")
    for k in range(12):
        nc.tensor.matmul(wacc[:], warm[:, 0:128], warm[:],
                         start=(k == 0), stop=(k == 11))

    # prologue: weights first on sync (small, unblock the first LDW), xg0
    # split right behind so the first chain starts on the first half; xg1
    # also on sync (SWDGE delivery measured too late for step 1)
    # exactly 8 prologue DMAs on the sync ring: one per DMA lane, so no
    # prologue tile's completion aliases with a later po-write on its lane
    # spread across all three rings: each HWDGE DMA pays ~1-2us of serial
    # completion receipt on its ring, so wk rides the otherwise-empty
    # gpsimd/SWDGE ring to land early for the k-chain
    nc.sync.dma_start(wq_s[:], wq[:])
    nc.gpsimd.dma_start(wk_s[:], wk[:])
    nc.sync.dma_start(wv_s[:], wv[:])
    emit_xg_load(0, nc.scalar)
    emit_xg_load(1, nc.scalar)
    nc.sync.dma_start(bqkv_s[:], bqkv[:])
    nc.sync.dma_start(trid_s[:], trid[:])
    nc.sync.dma_start(wo_s[:], wo[:])
    # ones columns 64:128 of V_aug (softmax denominator, replicated over the
    # 64 PSUM partitions 64:127 so normalize needs no partition broadcast).
    # memset, not DMA: a DMA's completion lands on a shared DMA-lane counter
    # that later po writes keep advancing, so every AV matmul would pick up
    # false waits on po-write completions.
    va_ones = vaug[:].rearrange("p c h x -> p (c h) x")[:, :, 64:128]
    nc.gpsimd.memset(va_ones, 1.0)
    emit_xg_load(2)
    for t in qkv_quanta(0):
        t()
    carry_outproj = []

    for idx, (b, j) in enumerate(steps):
        gg = b * GPB + j
        cols = slice(gg * 512, (gg + 1) * 512)

        # work to interleave into this step's attention waves: the previous
        # step's output projection (carried, so it fills wave slack instead
        # of bursting ahead of ACT), next step's QKV, and an xg prefetch
        from collections import deque as _dq
        quanta = _dq()
        if idx + 3 < len(steps):
            quanta.append(lambda i=idx + 3: emit_xg_load(i))
        quanta.extend(carry_outproj)
        carry_outproj = []
        if idx + 1 < len(steps):
            quanta.extend(qkv_quanta(idx + 1))
        n_waves = 4 * j + 3
        waves_left = [n_waves]   # mutable for the adaptive pop below
        last_step = (idx == len(steps) - 1)

        per_wave = max(1, -(-len(quanta) // n_waves))

        def pop_quanta():
            # front-load (keeps early waves dense for the HAM clock-gate)
            # but reserve one quantum per remaining wave so no wave is bare
            k = min(per_wave, max(1, len(quanta) - (waves_left[0] - 1)))
            for _ in range(k):
                if quanta:
                    quanta.popleft()()
            waves_left[0] -= 1

        # ---- attention blocks unlocked by this group
        for qi in (2 * j, 2 * j + 1):
            qcols = slice(b * S + qi * 256, b * S + qi * 256 + 256)
            nch = 2 * (qi + 1)          # kv chunks of 128 for this q block
            ct0 = psC.tile([128, 256], F32, tag="ct0")
            ct1 = psC.tile([128, 256], F32, tag="ct1")
            cts = [ct0, ct1]
            def av_wave(w, e):
                # AV accumulation for wave w (kv chunks 2w, 2w+1); alternate
                # heads so consecutive matmuls hit different PSUM banks
                for i, jj in enumerate((2 * w, 2 * w + 1)):
                    for h in range(HPC):
                        lhsT = vaug[:, b * (S // 128) + jj, h, :]
                        first = (jj == 0)
                        last = (jj == nch - 1)
                        if jj == nch - 2:  # even diag chunk: mask lower tri
                            nc.gpsimd.tensor_mul(
                                e[:, 2 * h + i, 0:128],
                                e[:, 2 * h + i, 0:128], tri_s[:])
                            nc.tensor.matmul(cts[h][:, :], lhsT,
                                             e[:, 2 * h + i, :],
                                             start=first, stop=last)
                        elif jj == nch - 1:  # odd diag: q first half masked
                            nc.gpsimd.tensor_mul(
                                e[:, 2 * h + i, 128:256],
                                e[:, 2 * h + i, 128:256], tri_s[:])
                            nc.tensor.matmul(cts[h][:, 128:256], lhsT,
                                             e[:, 2 * h + i, 128:256],
                                             start=first, stop=last)
                        else:
                            nc.tensor.matmul(cts[h][:, :], lhsT,
                                             e[:, 2 * h + i, :],
                                             start=first, stop=last)

            # software pipeline: emit scores(w)+exp(w) before AV(w-1), so the
            # PE instruction stream is S0 S1 A0 S2 A1 ... and exp overlaps AV
            prev_e = None
            for w in range(qi + 1):
                js = (2 * w, 2 * w + 1)
                sp = psS.tile([128, 2 * HPC, 256], SP_DT, tag="sp")
                for h in range(HPC):
                    for i, jj in enumerate(js):
                        kcols = slice(b * S + jj * 128,
                                      b * S + jj * 128 + 128)
                        nc.tensor.matmul(
                            sp[:, 2 * h + i, :],
                            kT[h * 64:(h + 1) * 64, kcols],
                            qT[h * 64:(h + 1) * 64, qcols],
                            start=True, stop=True)
                e = ep.tile([128, 2 * HPC, 256], MM_DT, tag="e")
                nc.scalar.activation(e[:], sp[:], EXPFN, scale=0.125)
                pop_quanta()
                if prev_e is not None:
                    av_wave(w - 1, prev_e)
                prev_e = e
            av_wave(qi, prev_e)
            # normalize: ctxT[:, q] = ct[0:64] * recip(ct[64:128]) - the
            # denominator is already replicated across partitions 64:127
            for h in range(HPC):
                # tensor_copy is the only DVE op that honors a partition
                # shift; custom-DVE (recip) and TT reads do not, so copy the
                # replicated denominators down to partitions 0:63 first
                d = rp.tile([64, 256], F32, tag="d")
                nc.vector.tensor_copy(d[:], cts[h][64:128, :])
                r = rp.tile([64, 256], F32, tag="r")
                nc.vector.reciprocal_approx_fast(r[:], d[:])
                nc.vector.tensor_mul(ctxT[h * 64:(h + 1) * 64, qcols],
                                     cts[h][0:64, :], r[:])
                if t_out_dbg is not None:
                    nc.vector.tensor_copy(
                        dbg_craw_s[h * 64:(h + 1) * 64, qcols],
                        cts[h][0:64, :])
                    nc.vector.tensor_copy(
                        dbg_den_s[h * 64:(h + 1) * 64, qcols],
                        cts[h][64:128, :])

            if last_step and qi == 2 * j:
                # feed the final block's waves with the first half's output
                # projection (no next-step QKV left to fill PE slack)
                half = slice(gg * 512, gg * 512 + 256)
                osth = osp.tile([128, KC, 256], MM_DT, tag="ost",
                                name="osth")
                for c in range(KC):
                    def proj_half(c=c):
                        op = psA.tile([128, 256], SP_DT, tag="accop",
                                      name="oph")
                        nc.tensor.matmul(op[:], woc[:, c, :], ctxT[:, half],
                                         start=True, stop=True)
                        nc.vector.tensor_copy(osth[:, c, :], op[:])
                        if c == KC - 1:
                            nc.sync.dma_start(po_pview[:, :, half], osth[:])
                    quanta.append(proj_half)

        while quanta:          # leftover next-step QKV work
            quanta.popleft()()

        # ---- output projection for the completed 512-token group
        # (the last step projected its first half inside the final block's
        # waves - see proj_half - so only the second half remains).
        # All 8 chunks stage into one SBUF tile and ship as a single DMA -
        # fewer DMA-lane updates means fewer conservative cross-waits.
        # Non-final steps defer the chunks into the next step's waves.
        pcols = slice(gg * 512 + 256, (gg + 1) * 512) if last_step else cols
        pn = 256 if last_step else 512
        ostb = osp.tile([128, KC, pn], MM_DT, tag="ost", name="ostb")

        def proj_chunk(c, pcols=pcols, pn=pn, ostb=ostb):
            op = psA.tile([128, pn], SP_DT, tag="accop", name="op")
            nc.tensor.matmul(op[:], woc[:, c, :], ctxT[:, pcols],
                             start=True, stop=True)
            nc.vector.tensor_copy(ostb[:, c, :], op[:])
            if c == KC - 1:
                nc.sync.dma_start(po_pview[:, :, pcols], ostb[:])

        if last_step:
            for c in range(KC):
                proj_chunk(c)
        else:
            carry_outproj = [lambda c=c: proj_chunk(c) for c in range(KC)]

    if t_out_dbg is not None:
        dq, dk, dc, dv, dcr, dde = t_out_dbg
        st = ctx.enter_context(tc.tile_pool(name="dbg", bufs=1))
        for src, dst in ((qT, dq), (kT, dk), (ctxT, dc)):
            tmp = st.tile([128, T], F32, tag="dbgt")
            nc.vector.tensor_copy(tmp[:], src[:])
            nc.sync.dma_start(dst[:], tmp[:])
        nc.sync.dma_start(dcr[:], dbg_craw_s[:])
        nc.sync.dma_start(dde[:], dbg_den_s[:])
        tmpv = st.tile([128, NKV * HPC * 128], F32, tag="dbgt")
        nc.vector.tensor_copy(
            tmpv[:], vaug[:].rearrange("p c h x -> p (c h x)"))
        nc.sync.dma_start(dv[:], tmpv[:])


_NC = None


def _build():
    global _NC
    if _NC is not None:
        return _NC
    nc = bacc.Bacc("TRN2", target_bir_lowering=False, debug=False,
                   num_devices=NCORES)
    t_in = [
        nc.dram_tensor("xt", [NG, 128, KC, 512], MM_DT, kind="ExternalInput").ap(),
        nc.dram_tensor("wq", [128, KC, 128], MM_DT, kind="ExternalInput").ap(),
        nc.dram_tensor("wk", [128, KC, 128], MM_DT, kind="ExternalInput").ap(),
        nc.dram_tensor("wv", [128, KC, 128], MM_DT, kind="ExternalInput").ap(),
        nc.dram_tensor("wo", [128, D], MM_DT, kind="ExternalInput").ap(),
        nc.dram_tensor("bqkv", [128, 3], F32, kind="ExternalInput").ap(),
        nc.dram_tensor("trid", [128, 2, 128], MM_DT, kind="ExternalInput").ap(),
    ]
    po = nc.dram_tensor("po", [KC, 128, T], MM_DT, kind="ExternalOutput").ap()
    t_out_dbg = None
    if os.environ.get("KERNEL_DEBUG_TAPS") == "1":
        t_out_dbg = [
            nc.dram_tensor("dbg_qT", [128, T], F32, kind="ExternalOutput").ap(),
            nc.dram_tensor("dbg_kT", [128, T], F32, kind="ExternalOutput").ap(),
            nc.dram_tensor("dbg_ctxT", [128, T], F32, kind="ExternalOutput").ap(),
            nc.dram_tensor("dbg_vaug", [128, NKV * HPC * 128], F32,
                           kind="ExternalOutput").ap(),
            nc.dram_tensor("dbg_craw", [128, T], F32, kind="ExternalOutput").ap(),
            nc.dram_tensor("dbg_den", [128, T], F32, kind="ExternalOutput").ap(),
        ]
    with tile.TileContext(nc) as tc, ExitStack() as ctx:
        _body(nc, tc, ctx, t_in, po, t_out_dbg)
    nc.compile()
    _NC = nc
    return nc


def _in_maps(hidden_states, Wq, bq, Wk, bk, Wv, bv, Wo, bo):
    hid = np.asarray(hidden_states, dtype=np.float32).reshape(T, D)
    hidT = hid.T.astype(MM_NP)                       # [D, T]
    xt = np.ascontiguousarray(
        hidT.reshape(KC, 128, NG, 512).transpose(2, 1, 0, 3))
    common = {
        "xt": xt,
        "trid": np.ascontiguousarray(np.stack(
            [np.triu(np.ones((128, 128), MM_NP)),
             np.eye(128, dtype=MM_NP)], axis=1)),
    }
    maps = []
    for c in range(NCORES):
        cs = slice(c * 128, (c + 1) * 128)
        maps.append(dict(
            common,
            wq=np.ascontiguousarray(np.asarray(Wq)[:, cs].astype(MM_NP).reshape(KC, 128, 128).transpose(1, 0, 2)),
            wk=np.ascontiguousarray(np.asarray(Wk)[:, cs].astype(MM_NP).reshape(KC, 128, 128).transpose(1, 0, 2)),
            wv=np.ascontiguousarray(np.asarray(Wv)[:, cs].astype(MM_NP).reshape(KC, 128, 128).transpose(1, 0, 2)),
            wo=np.ascontiguousarray(np.asarray(Wo)[cs, :].astype(MM_NP)),
            bqkv=np.ascontiguousarray(np.stack(
                [np.asarray(bq)[cs], np.asarray(bk)[cs],
                 np.asarray(bv)[cs]], axis=1).astype(np.float32)),
        ))
    return maps


def kernel(hidden_states, Wq, bq, Wk, bk, Wv, bv, Wo, bo):
    nc = _build()
    maps = _in_maps(hidden_states, Wq, bq, Wk, bk, Wv, bv, Wo, bo)
    res = run_bass_kernel_spmd(nc, maps, list(range(NCORES))).results
    acc = np.zeros((KC, 128, T), np.float32)
    for r in res:
        acc += r["po"].astype(np.float32)
    outT = acc.reshape(D, T)
    out = outT.T + np.asarray(bo, dtype=np.float32)[None, :]
    return out.reshape(B, S, D).astype(np.float32)
